# revision 6
# baseline (speedup 1.0000x reference)
"""Trainium2 Bass kernel for nn_DiffusionModel (theta_post_prob).

Math (per batch b, runtime scalars a = alphas-gather, ca = cumalphas-gather,
k1 = (1-a)/C, u = (1-ca)/C, M = ca*I + u*ones):
    p     = a*xt + k1
    denom = M^T p = a*(M^T xt) + k1          (column sums of M are 1)
    out   = p * (M (theta_x0 / denom))

All three HBM tensors travel as 16-bit (inputs fp16, output bf16), halving
memory traffic vs fp32. Power-of-2 scale factors (exact in binary fp) keep
every fp16 value inside the normal range:
    xs = S*xt, ys = S*x0                      S = 4096
    ma = MSeff*(a*M + k1*ones)   (fp16)       MSeff = 2^ceil(log2(2e-4/dmin)),
                                              dmin = u + ca*k1 <= min denom;
                                              the k1*ones fold works because
                                              classes sum to 1 per pixel
    dn   = ma^T xs = MSeff*S*denom            (PSUM fp32)
    rden = 1/dn                               (DVE reciprocal_approx_fast)
    g    = ys * rden = x0/(MSeff*denom)       (fp16; MSeff simultaneously
                                              keeps ma >= 2e-4 and g <= ~1e4)
    r    = mb^T g,  mb = MS2*M                MS2 = 4096 (PSUM fp32, staged to
                                              SBUF bf16 by the Act engine)
    o    = (alpha*xs + beta) * r = out        alpha = a*MSeff/(S*MS2),
                                              beta = k1*MSeff/MS2; bf16 store
                                              (out in (0,1], sums to 1 -> no
                                              range cliff in bf16)

Kernel layout: batch b -> core b (pure data parallel, 8 cores). Per core the
(C=32, HW=65536) slab is processed as [128, N] tiles where the 128 partitions
pack G=4 spatial blocks x 32 classes; both class-reductions are PE matmuls
against block-diagonal 128x128 fp16 matrices kron(.., I4) built on host.

Structure: two passes per DMA tile so each engine gets long runs of one op
type (pipelines across chunks without head-of-line blocking):
  pass A: mm1 chunks back-to-back, DVE reciprocal into a full-tile buffer
  pass B: Pool g-mult, mm2, Act r->bf16 stage, DVE fused (alpha*xs+beta)*r
Engine constraints found on HW: GPSIMD cannot access PSUM; tensor_tensor
divide and scalar_tensor_tensor are not valid Pool/DVE ISA ops; activation
Reciprocal is banned (accuracy); so the division = DVE recip + Pool mult.
The Act r-copy makes every affine operand 16-bit SBUF (DVE 2x mode).
"""

import math
import os
import sys

if "/opt/trn_rl_repo" not in sys.path:
    sys.path.insert(0, "/opt/trn_rl_repo")

import numpy as np

import concourse.bacc as bacc
import concourse.mybir as mybir
from concourse.tile import TileContext
from concourse.bass_utils import run_bass_kernel_spmd

F16 = mybir.dt.float16
F32 = mybir.dt.float32
BF16 = mybir.dt.bfloat16

T = 1000
C = 32
B = 8
H = 256
W = 256
HW = H * W

NCORES = 8
G = 4                 # spatial blocks packed into the 128 partitions
P = G * C             # 128
COLS = HW // G        # 16384 columns per spatial block
MM_N = 512            # max moving free-dim per matmul (PSUM bank)

S = 4096.0            # input scale (power of 2, exact)
MS2 = 4096.0          # mb scale
TH = 2e-4             # min-normal target for ma entries (>> 6e-5 cliff)


def _cfg():
    return {
        "ntl": int(os.environ.get("KCFG_NTL", "2048")),   # DMA tile width
        "nc": int(os.environ.get("KCFG_NC", "512")),      # PSUM tile width
        "rcpy": os.environ.get("KCFG_RCPY", "act"),       # act | off  r PSUM->bf16
        "gsplit": int(os.environ.get("KCFG_GSPLIT", "0")),  # of every 8 g-chunks, run this many on DVE
        "ldbufs": int(os.environ.get("KCFG_LDBUFS", "5")),
        "gbufs": int(os.environ.get("KCFG_GBUFS", "4")),
        "rdbufs": int(os.environ.get("KCFG_RDBUFS", "2")),
        "psbufs": int(os.environ.get("KCFG_PSBUFS", "4")),
        "xsrc": os.environ.get("KCFG_XSRC", "sp"),        # sp | act  x-load ring
        "ysrc": os.environ.get("KCFG_YSRC", "act"),       # sp | act  y-load ring
        "store": os.environ.get("KCFG_STORE", "sp"),      # pool | sp | act
    }


_CACHE = {}


def _build():
    cfg = _cfg()
    key = tuple(sorted(cfg.items()))
    if key in _CACHE:
        return _CACHE[key]

    NTL = cfg["ntl"]
    NC = cfg["nc"]
    RCPY = cfg["rcpy"] == "act"
    assert NTL % NC == 0 and NC % MM_N == 0
    NMM = NC // MM_N
    NCH = NTL // NC

    nc = bacc.Bacc(
        "TRN2",
        target_bir_lowering=False,
        debug=False,
        enable_asserts=False,
        num_devices=NCORES,
    )

    xs_d = nc.dram_tensor("xs", [P, COLS], F16, kind="ExternalInput")
    ys_d = nc.dram_tensor("ys", [P, COLS], F16, kind="ExternalInput")
    ma_d = nc.dram_tensor("ma", [P, P], F16, kind="ExternalInput")
    mb_d = nc.dram_tensor("mb", [P, P], F16, kind="ExternalInput")
    sc_d = nc.dram_tensor("sc", [P, 2], F32, kind="ExternalInput")
    out_d = nc.dram_tensor("out", [P, COLS], BF16, kind="ExternalOutput")

    AF = mybir.ActivationFunctionType
    ALU = mybir.AluOpType
    store_eng = {"pool": nc.gpsimd, "sp": nc.sync, "act": nc.scalar}[cfg["store"]]
    x_eng = {"sp": nc.sync, "act": nc.scalar}[cfg["xsrc"]]
    y_eng = {"sp": nc.sync, "act": nc.scalar}[cfg["ysrc"]]

    with TileContext(nc) as tc:
        with (
            tc.tile_pool(name="consts", bufs=1) as cpool,
            tc.tile_pool(name="work", bufs=4) as pool,
            tc.tile_pool(name="psum", bufs=cfg["psbufs"], space="PSUM") as psum,
        ):
            ma = cpool.tile([P, P], F16)
            nc.sync.dma_start(ma[:, :], ma_d[:, :])
            mb = cpool.tile([P, P], F16)
            nc.sync.dma_start(mb[:, :], mb_d[:, :])
            sc = cpool.tile([P, 2], F32)
            nc.sync.dma_start(sc[:, :], sc_d[:, :])
            al_col = sc[:, 0:1]    # a*MSeff/(S*MS2)
            be_col = sc[:, 1:2]    # k1*MSeff/MS2

            gchunk = 0
            off = 0
            for i in range(COLS // NTL):
                sl = slice(off, off + NTL)
                x = pool.tile([P, NTL], F16, bufs=cfg["ldbufs"], tag="x",
                              name=f"x_{i}")
                x_eng.dma_start(x[:, :], xs_d[:, sl])
                y = pool.tile([P, NTL], F16, bufs=cfg["ldbufs"], tag="y",
                              name=f"y_{i}")
                y_eng.dma_start(y[:, :], ys_d[:, sl])
                o = pool.tile([P, NTL], BF16, bufs=cfg["ldbufs"], tag="o",
                              name=f"o_{i}")

                # pass A: dn = kron(MSeff*(a*M + k1), I4)^T @ xs
                # (= MSeff*S*denom), then rden = 1/dn into a full-tile buffer
                rden = pool.tile([P, NTL], F32, bufs=cfg["rdbufs"],
                                 tag="rden", name=f"rden_{i}")
                for c in range(NCH):
                    dn = psum.tile([P, NC], F32, tag="dn", name=f"dn_{i}_{c}")
                    for m in range(NMM):
                        ms = slice(m * MM_N, (m + 1) * MM_N)
                        nc.tensor.matmul(dn[:, ms], ma[:, :],
                                         x[:, c * NC + m * MM_N:
                                           c * NC + (m + 1) * MM_N],
                                         start=True, stop=True)
                    cs = slice(c * NC, (c + 1) * NC)
                    nc.vector.reciprocal_approx_fast(out=rden[:, cs],
                                                     in_=dn[:, :])

                # pass B: g = ys*rden (fp16), r = kron(MS2*M, I4)^T @ g,
                #         o = (alpha*xs + beta)*r  (bf16)
                for c in range(NCH):
                    cs = slice(c * NC, (c + 1) * NC)
                    g = pool.tile([P, NC], F16, bufs=cfg["gbufs"], tag="g",
                                  name=f"g_{i}_{c}")
                    g_eng = nc.vector if gchunk % 8 < cfg["gsplit"] else nc.gpsimd
                    g_eng.tensor_tensor(g[:, :], y[:, cs], rden[:, cs],
                                        ALU.mult)
                    gchunk += 1

                    r = psum.tile([P, NC], F32, tag="r", name=f"r_{i}_{c}")
                    for m in range(NMM):
                        ms = slice(m * MM_N, (m + 1) * MM_N)
                        nc.tensor.matmul(r[:, ms], mb[:, :],
                                         g[:, m * MM_N:(m + 1) * MM_N],
                                         start=True, stop=True)

                    if RCPY:
                        r16 = pool.tile([P, NC], BF16, bufs=cfg["gbufs"],
                                        tag="r16", name=f"r16_{i}_{c}")
                        nc.scalar.activation(r16[:, :], r[:, :], AF.Copy)
                        r_in = r16[:, :]
                    else:
                        r_in = r[:, :]

                    acc = pool.tile([P, 1], F32, tag="acc", name=f"acc_{i}_{c}")
                    nc.vector.affine_mul_reduce(
                        out=o[:, cs], accum_out=acc[:, :], in0=x[:, cs],
                        in1=r_in, scale=al_col, bias=be_col)

                store_eng.dma_start(out_d[:, sl], o[:, :])
                off += NTL

    nc.compile()
    _CACHE[key] = nc
    return nc


def _host_prep(inputs):
    xt = np.asarray(inputs["xt"], dtype=np.float32)
    x0 = np.asarray(inputs["theta_x0"], dtype=np.float32)
    t = np.asarray(inputs["t"]).astype(np.int64)
    al = np.asarray(inputs["alphas"], dtype=np.float64)
    cu = np.asarray(inputs["cumalphas"], dtype=np.float64)

    eyeC = np.eye(C, dtype=np.float64)
    eyeG = np.eye(G, dtype=np.float64)
    in_maps = []
    for b in range(B):
        tm = int(t[b]) - 1
        a = 0.0 if tm == 0 else float(al[tm])
        ca = 1.0 if tm == 0 else float(cu[tm - 1])
        u = (1.0 - ca) / C
        k1 = (1.0 - a) / C
        M = ca * eyeC + u

        # per-batch power-of-2 scale: keeps ma entries >= ~2e-4 (fp16 normal)
        # and g = x0/(MSeff*denom) <= ~1e4 (fp16 max) at the same time
        dmin = u + ca * k1                      # denom >= dmin (xt >= 0)
        MSeff = 2.0 ** math.ceil(math.log2(TH / dmin))

        ma = np.kron(MSeff * (a * M + k1), eyeG).astype(np.float16)
        mb = np.kron(MS2 * M, eyeG).astype(np.float16)
        sc = np.empty((P, 2), dtype=np.float32)
        sc[:, 0] = a * MSeff / (S * MS2)
        sc[:, 1] = k1 * MSeff / MS2
        in_maps.append(
            {
                "xs": (xt[b].reshape(P, COLS) * np.float32(S)).astype(np.float16),
                "ys": (x0[b].reshape(P, COLS) * np.float32(S)).astype(np.float16),
                "ma": ma,
                "mb": mb,
                "sc": sc,
            }
        )
    return in_maps


def _run(inputs, trace=False, **kw):
    nc = _build()
    in_maps = _host_prep(inputs)
    res = run_bass_kernel_spmd(
        nc, in_maps, core_ids=list(range(NCORES)), trace=trace, **kw
    )
    out = np.stack(
        [r["out"].astype(np.float32).reshape(C, H, W) for r in res.results]
    )
    return out, res


def kernel(**inputs):
    out, _ = _run(inputs, trace=False)
    return out
